# revision 2
# baseline (speedup 1.0000x reference)
"""Trainium2 Bass kernel for MoE-MLP (shared GeGLU base + top-1 LoRA experts).

Sharding: tokens (B*S = 8192) split evenly across 8 NeuronCores (1024 each);
MLP weights replicated per core (streamed from HBM, bf16). Router runs in
fp32 on-device for exact top-1; the tiny softmax/aux-loss reduction is
finished on host from the device-computed logits during unsharding.
"""

import numpy as np
import ml_dtypes
from contextlib import ExitStack

import concourse.bass as bass
import concourse.mybir as mybir
from concourse import bacc
from concourse.tile import TileContext
from concourse.bass_utils import run_bass_kernel_spmd
from concourse.masks import make_identity

P = 128
B, S, H, I = 4, 2048, 2048, 16384
NE = 5               # experts (1 base + 4 lora)
ER = 64              # lora (expert, rank) flattened
NT = B * S           # 8192 tokens
NCORES = 8
TC = NT // NCORES    # 1024 tokens per core
TPASS = 512          # tokens per pass (SBUF budget)
NPASS = TC // TPASS  # 2
ICH = 512            # intermediate-dim chunk
NIC = I // ICH       # 32
HSUB = H // P        # 16
NTT = TC // P        # 8 token tiles per core
TLP = TPASS // P     # 4 token tiles per pass
ITC = ICH // P       # 4 i-subtiles per chunk
NHC = H // 512       # 4 output column chunks

f32 = mybir.dt.float32
bf16 = mybir.dt.bfloat16
bf16_np = ml_dtypes.bfloat16

_CACHE = {}


def _build():
    nc = bacc.Bacc("TRN2", target_bir_lowering=False, debug=False)

    xT32 = nc.declare_dram_parameter("xT32", [H, TC], f32, isOutput=False)
    xT16 = nc.declare_dram_parameter("xT16", [H, TC], bf16, isOutput=False)
    gateT = nc.declare_dram_parameter("gateT", [H, I], bf16, isOutput=False)
    upT = nc.declare_dram_parameter("upT", [H, I], bf16, isOutput=False)
    downT = nc.declare_dram_parameter("downT", [I, H], bf16, isOutput=False)
    loraAT = nc.declare_dram_parameter("loraAT", [H, ER], bf16, isOutput=False)
    bflat = nc.declare_dram_parameter("bflat", [P, H], bf16, isOutput=False)
    routerT = nc.declare_dram_parameter("routerT", [H, NE], f32, isOutput=False)
    out = nc.declare_dram_parameter("out", [TC, H], f32, isOutput=True)
    logits = nc.declare_dram_parameter("logits", [TC, NE], f32, isOutput=True)

    xT32_r = xT32.rearrange("(ho hi) t -> hi ho t", hi=P)      # [128,16,1024]
    xT16_r = xT16.rearrange("(ho hi) t -> hi ho t", hi=P)
    gateT_r = gateT.rearrange("(ho hi) i -> hi ho i", hi=P)    # [128,16,16384]
    upT_r = upT.rearrange("(ho hi) i -> hi ho i", hi=P)
    downT_r = downT.rearrange("(io ii) h -> ii io h", ii=P)    # [128,128,2048]
    loraAT_r = loraAT.rearrange("(ho hi) e -> hi ho e", hi=P)  # [128,16,64]
    routerT_r = routerT.rearrange("(ho hi) e -> hi ho e", hi=P)
    out_r = out.rearrange("(to ti) h -> ti to h", ti=P)        # [128,8,2048]
    logits_r = logits.rearrange("(to ti) e -> ti to e", ti=P)  # [128,8,5]

    gelu = mybir.ActivationFunctionType.Gelu_apprx_tanh

    with TileContext(nc) as tc, ExitStack() as ctx:
        const = ctx.enter_context(tc.tile_pool(name="const", bufs=1))
        rpool = ctx.enter_context(tc.tile_pool(name="rpool", bufs=2))
        small = ctx.enter_context(tc.tile_pool(name="small", bufs=3))
        xpool = ctx.enter_context(tc.tile_pool(name="xpool", bufs=1))
        wpool = ctx.enter_context(tc.tile_pool(name="wpool", bufs=2))
        hpool = ctx.enter_context(tc.tile_pool(name="hpool", bufs=2))
        apool = ctx.enter_context(tc.tile_pool(name="apool", bufs=3))
        opool = ctx.enter_context(tc.tile_pool(name="opool", bufs=1))
        ps_small = ctx.enter_context(tc.tile_pool(name="ps_small", bufs=2, space="PSUM"))
        ps_g = ctx.enter_context(tc.tile_pool(name="ps_g", bufs=2, space="PSUM"))
        ps_u = ctx.enter_context(tc.tile_pool(name="ps_u", bufs=2, space="PSUM"))
        ps_o = ctx.enter_context(tc.tile_pool(name="ps_o", bufs=2, space="PSUM"))

        ident = const.tile([P, P], bf16)
        make_identity(nc, ident)

        routerT_sb = const.tile([P, HSUB, NE], f32)
        nc.sync.dma_start(routerT_sb[:], routerT_r[:, :, :])
        loraAT_sb = const.tile([P, HSUB, ER], bf16)
        nc.sync.dma_start(loraAT_sb[:], loraAT_r[:, :, :])
        bflat_sb = const.tile([P, H], bf16)
        nc.sync.dma_start(bflat_sb[:], bflat[:, :])
        tmaskT_sb = const.tile([P, NTT, P], bf16)
        nc.vector.memset(tmaskT_sb[:], 0.0)

        # ---- Phase A: router (fp32) + masked LoRA intermediate, per token tile
        for tt in range(NTT):
            xf = rpool.tile([P, HSUB, P], f32, tag="xf")
            nc.sync.dma_start(xf[:], xT32_r[:, :, tt * P:(tt + 1) * P])
            xb = rpool.tile([P, HSUB, P], bf16, tag="xb")
            nc.vector.tensor_copy(xb[:], xf[:])

            psl = ps_small.tile([P, P], f32, tag="pss")
            for hs in range(HSUB):
                nc.tensor.matmul(psl[:, :NE], xf[:, hs, :], routerT_sb[:, hs, :],
                                 start=(hs == 0), stop=(hs == HSUB - 1))
            lsb = small.tile([P, NE], f32, tag="lsb")
            nc.scalar.copy(lsb[:], psl[:, :NE])
            nc.sync.dma_start(logits_r[:, tt, :], lsb[:])
            mx = small.tile([P, 1], f32, tag="mx")
            nc.vector.reduce_max(mx[:], psl[:, :NE], axis=mybir.AxisListType.X)
            eq = small.tile([P, NE], bf16, tag="eq")
            nc.vector.tensor_tensor(eq[:], psl[:, :NE], mx[:].to_broadcast((P, NE)),
                                    op=mybir.AluOpType.is_equal)

            pst = ps_small.tile([P, P], f32, tag="pss")
            for hs in range(HSUB):
                nc.tensor.matmul(pst[:, :ER], xb[:, hs, :], loraAT_sb[:, hs, :],
                                 start=(hs == 0), stop=(hs == HSUB - 1))
            tmask = small.tile([P, 4, 16], bf16, tag="tmask")
            nc.vector.tensor_tensor(
                tmask[:],
                pst[:, :ER].rearrange("p (e r) -> p e r", r=16),
                eq[:, 1:NE, None].to_broadcast((P, 4, 16)),
                op=mybir.AluOpType.mult,
            )
            pstr = ps_small.tile([P, P], bf16, tag="pss")
            nc.tensor.transpose(pstr[:ER, :], tmask[:].rearrange("p e r -> p (e r)"),
                                ident[:])
            nc.vector.tensor_copy(tmaskT_sb[:ER, tt, :], pstr[:ER, :])

        # ---- Phase B: GeGLU base MLP (bf16) + LoRA output, two token passes
        for tp in range(NPASS):
            xt = xpool.tile([P, HSUB, TPASS], bf16, tag="xt")
            nc.sync.dma_start(xt[:], xT16_r[:, :, tp * TPASS:(tp + 1) * TPASS])
            out_sb = opool.tile([P, TLP, H], f32, tag="out_sb")

            for ic in range(NIC):
                gt = wpool.tile([P, HSUB, ICH], bf16, tag="gt")
                nc.sync.dma_start(gt[:], gateT_r[:, :, ic * ICH:(ic + 1) * ICH])
                ut = wpool.tile([P, HSUB, ICH], bf16, tag="ut")
                nc.sync.dma_start(ut[:], upT_r[:, :, ic * ICH:(ic + 1) * ICH])
                dt_ = wpool.tile([P, ITC, H], bf16, tag="dt")
                nc.sync.dma_start(dt_[:], downT_r[:, ic * ITC:(ic + 1) * ITC, :])

                hh = hpool.tile([P, ITC, TPASS], bf16, tag="hh")
                for it in range(ITC):
                    psg = ps_g.tile([P, TPASS], f32, tag="psg")
                    for hs in range(HSUB):
                        nc.tensor.matmul(psg[:], gt[:, hs, it * P:(it + 1) * P],
                                         xt[:, hs, :],
                                         start=(hs == 0), stop=(hs == HSUB - 1))
                    psu = ps_u.tile([P, TPASS], f32, tag="psu")
                    for hs in range(HSUB):
                        nc.tensor.matmul(psu[:], ut[:, hs, it * P:(it + 1) * P],
                                         xt[:, hs, :],
                                         start=(hs == 0), stop=(hs == HSUB - 1))
                    ga = apool.tile([P, TPASS], bf16, tag="ga")
                    nc.scalar.activation(ga[:], psg[:], gelu)
                    nc.vector.tensor_mul(hh[:, it, :], ga[:], psu[:])

                for tl in range(TLP):
                    for hc in range(NHC):
                        pso = ps_o.tile([P, 512], f32, tag="pso")
                        for it in range(ITC):
                            nc.tensor.matmul(pso[:], hh[:, it, tl * P:(tl + 1) * P],
                                             dt_[:, it, hc * 512:(hc + 1) * 512],
                                             start=(it == 0), stop=(it == ITC - 1))
                        osl = out_sb[:, tl, hc * 512:(hc + 1) * 512]
                        if ic == 0:
                            nc.scalar.copy(osl, pso[:])
                        else:
                            nc.vector.tensor_add(osl, osl, pso[:])

            for tl in range(TLP):
                tt = tp * TLP + tl
                for hc in range(NHC):
                    pso = ps_o.tile([P, 512], f32, tag="pso")
                    nc.tensor.matmul(pso[:], tmaskT_sb[:, tt, :],
                                     bflat_sb[:, hc * 512:(hc + 1) * 512],
                                     start=True, stop=True)
                    osl = out_sb[:, tl, hc * 512:(hc + 1) * 512]
                    nc.vector.tensor_add(osl, osl, pso[:])
                nc.sync.dma_start(out_r[:, tp * TLP + tl, :], out_sb[:, tl, :])

    nc.compile()
    return nc


def _get_nc():
    if "nc" not in _CACHE:
        _CACHE["nc"] = _build()
    return _CACHE["nc"]


def _prep_inputs(hidden_states, router_w, gate_w, up_w, down_w, lora_A, lora_B):
    x = np.ascontiguousarray(hidden_states, dtype=np.float32).reshape(NT, H)

    gateT = np.ascontiguousarray(gate_w.astype(bf16_np).T)      # [H, I]
    upT = np.ascontiguousarray(up_w.astype(bf16_np).T)          # [H, I]
    downT = np.ascontiguousarray(down_w.astype(bf16_np).T)      # [I, H]
    loraAT = np.ascontiguousarray(
        lora_A.reshape(ER, H).astype(bf16_np).T)                # [H, 64]
    bflat = np.zeros((P, H), dtype=bf16_np)
    bflat[:ER] = (lora_B.transpose(0, 2, 1).reshape(ER, H) * 2.0).astype(bf16_np)
    routerT = np.ascontiguousarray(router_w.T, dtype=np.float32)  # [H, 5]

    in_maps = []
    for c in range(NCORES):
        xc = np.ascontiguousarray(x[c * TC:(c + 1) * TC].T)     # [H, TC]
        in_maps.append({
            "xT32": xc,
            "xT16": xc.astype(bf16_np),
            "gateT": gateT,
            "upT": upT,
            "downT": downT,
            "loraAT": loraAT,
            "bflat": bflat,
            "routerT": routerT,
        })
    return in_maps


def run(inputs, trace=False, **trace_kwargs):
    """Compile (cached), shard, run on 8 cores; returns (output, aux_loss, raw)."""
    nc = _get_nc()
    in_maps = _prep_inputs(**inputs)
    raw = run_bass_kernel_spmd(nc, in_maps, list(range(NCORES)), trace=trace,
                               **trace_kwargs)

    out = np.empty((NT, H), dtype=np.float32)
    logits = np.empty((NT, NE), dtype=np.float32)
    for c, r in enumerate(raw.results):
        out[c * TC:(c + 1) * TC] = r["out"]
        logits[c * TC:(c + 1) * TC] = r["logits"]

    # aux loss from device-computed fp32 logits (tiny O(NT*E) reduction)
    m = logits.max(axis=1, keepdims=True)
    p = np.exp(logits - m)
    p /= p.sum(axis=1, keepdims=True)
    expert_probs = p.mean(axis=0)                                # [5]
    sel = logits.argmax(axis=1)
    counts = np.bincount(sel, minlength=NE).astype(np.float32) / NT
    aux = np.float32((expert_probs * counts).sum() * NE * 0.01)

    return out.reshape(B, S, H), aux, raw


def kernel(**inputs):
    output, aux, _ = run(inputs, trace=False)
    return output, aux


# revision 8
# speedup vs baseline: 1.0011x; 1.0011x over previous
"""Trainium2 Bass kernel for MoE-MLP (shared GeGLU base + top-1 LoRA experts).

Sharding: tokens (B*S = 8192) split evenly across 8 NeuronCores (1024 each);
MLP weights replicated per core (streamed from HBM, bf16). Router runs in
fp32 on-device for exact top-1; the tiny softmax/aux-loss reduction is
finished on host from the device-computed logits during unsharding.
"""

import numpy as np
import ml_dtypes
from contextlib import ExitStack

import concourse.bass as bass
import concourse.mybir as mybir
from concourse import bacc
from concourse.tile import TileContext
from concourse.bass_utils import run_bass_kernel_spmd
from concourse.masks import make_identity

P = 128
B, S, H, I = 4, 2048, 2048, 16384
NE = 5               # experts (1 base + 4 lora)
ER = 64              # lora (expert, rank) flattened
NT = B * S           # 8192 tokens
NCORES = 8
TC = NT // NCORES    # 1024 tokens per core
TPASS = 512          # tokens per pass (SBUF budget)
NPASS = TC // TPASS  # 2
ICH = 512            # intermediate-dim chunk
NIC = I // ICH       # 32
HSUB = H // P        # 16
NTT = TC // P        # 8 token tiles per core
TLP = TPASS // P     # 4 token tiles per pass
ITC = ICH // P       # 4 i-subtiles per chunk
NHC = H // 512       # 4 output column chunks

f32 = mybir.dt.float32
bf16 = mybir.dt.bfloat16
bf16_np = ml_dtypes.bfloat16

_CACHE = {}


def _build():
    nc = bacc.Bacc("TRN2", target_bir_lowering=False, debug=False)

    xT16 = nc.declare_dram_parameter("xT16", [H, TC], bf16, isOutput=False)
    xloT = nc.declare_dram_parameter("xloT", [H, TC], bf16, isOutput=False)
    gateT = nc.declare_dram_parameter("gateT", [H, I], bf16, isOutput=False)
    upT = nc.declare_dram_parameter("upT", [H, I], bf16, isOutput=False)
    downT = nc.declare_dram_parameter("downT", [I, H], bf16, isOutput=False)
    loraAT = nc.declare_dram_parameter("loraAT", [H, ER], bf16, isOutput=False)
    bflat = nc.declare_dram_parameter("bflat", [P, H], bf16, isOutput=False)
    rhi = nc.declare_dram_parameter("rhi", [H, NE], bf16, isOutput=False)
    rlo = nc.declare_dram_parameter("rlo", [H, NE], bf16, isOutput=False)
    out = nc.declare_dram_parameter("out", [TC, H], f32, isOutput=True)
    logits = nc.declare_dram_parameter("logits", [TC, NE], f32, isOutput=True)

    xT16_r = xT16.rearrange("(ho hi) t -> hi ho t", hi=P)      # [128,16,1024]
    xloT_r = xloT.rearrange("(ho hi) t -> hi ho t", hi=P)
    gateT_r = gateT.rearrange("(ho hi) i -> hi ho i", hi=P)    # [128,16,16384]
    upT_r = upT.rearrange("(ho hi) i -> hi ho i", hi=P)
    downT_r = downT.rearrange("(io ii) h -> ii io h", ii=P)    # [128,128,2048]
    loraAT_r = loraAT.rearrange("(ho hi) e -> hi ho e", hi=P)  # [128,16,64]
    rhi_r = rhi.rearrange("(ho hi) e -> hi ho e", hi=P)
    rlo_r = rlo.rearrange("(ho hi) e -> hi ho e", hi=P)
    out_r = out.rearrange("(to ti) h -> ti to h", ti=P)        # [128,8,2048]
    logits_r = logits.rearrange("(to ti) e -> ti to e", ti=P)  # [128,8,5]

    gelu = mybir.ActivationFunctionType.Gelu_apprx_tanh

    with TileContext(nc) as tc, ExitStack() as ctx:
        const = ctx.enter_context(tc.tile_pool(name="const", bufs=1))
        rpool = ctx.enter_context(tc.tile_pool(name="rpool", bufs=2))
        small = ctx.enter_context(tc.tile_pool(name="small", bufs=3))
        xpool = ctx.enter_context(tc.tile_pool(name="xpool", bufs=2))
        wpool = ctx.enter_context(tc.tile_pool(name="wpool", bufs=2))
        hpool = ctx.enter_context(tc.tile_pool(name="hpool", bufs=2))
        apool = ctx.enter_context(tc.tile_pool(name="apool", bufs=3))
        opool = ctx.enter_context(tc.tile_pool(name="opool", bufs=1))
        ps_small = ctx.enter_context(tc.tile_pool(name="ps_small", bufs=2, space="PSUM"))
        ps_g = ctx.enter_context(tc.tile_pool(name="ps_g", bufs=2, space="PSUM"))
        ps_u = ctx.enter_context(tc.tile_pool(name="ps_u", bufs=2, space="PSUM"))
        ps_o = ctx.enter_context(tc.tile_pool(name="ps_o", bufs=2, space="PSUM"))

        ident = const.tile([P, P], bf16)
        make_identity(nc, ident)

        rhi_sb = const.tile([P, HSUB, NE], bf16)
        nc.sync.dma_start(rhi_sb[:], rhi_r[:, :, :])
        rlo_sb = const.tile([P, HSUB, NE], bf16)
        nc.sync.dma_start(rlo_sb[:], rlo_r[:, :, :])
        loraAT_sb = const.tile([P, HSUB, ER], bf16)
        nc.sync.dma_start(loraAT_sb[:], loraAT_r[:, :, :])
        bflat_sb = const.tile([P, H], bf16)
        nc.sync.dma_start(bflat_sb[:], bflat[:, :])
        tmaskT_sb = const.tile([P, NTT, P], bf16)
        nc.vector.memset(tmaskT_sb[:], 0.0)

        # ---- Phase A: router (compensated bf16: xb@rhi + xb@rlo + xlo@rhi,
        # logits error ~1e-5 => argmax matches fp32) + masked LoRA intermediate
        for tt in range(NTT):
            xb = rpool.tile([P, HSUB, P], bf16, tag="xb")
            nc.sync.dma_start(xb[:], xT16_r[:, :, tt * P:(tt + 1) * P])
            xlo = rpool.tile([P, HSUB, P], bf16, tag="xlo")
            nc.sync.dma_start(xlo[:], xloT_r[:, :, tt * P:(tt + 1) * P])

            psl = ps_small.tile([P, P], f32, tag="pss")
            for hs in range(HSUB):
                nc.tensor.matmul(psl[:, :NE], xb[:, hs, :], rhi_sb[:, hs, :],
                                 start=(hs == 0), stop=False)
                nc.tensor.matmul(psl[:, :NE], xb[:, hs, :], rlo_sb[:, hs, :],
                                 start=False, stop=False)
                nc.tensor.matmul(psl[:, :NE], xlo[:, hs, :], rhi_sb[:, hs, :],
                                 start=False, stop=(hs == HSUB - 1))
            lsb = small.tile([P, NE], f32, tag="lsb")
            nc.scalar.copy(lsb[:], psl[:, :NE])
            nc.sync.dma_start(logits_r[:, tt, :], lsb[:])
            mx = small.tile([P, 1], f32, tag="mx")
            nc.vector.reduce_max(mx[:], psl[:, :NE], axis=mybir.AxisListType.X)
            eq = small.tile([P, NE], bf16, tag="eq")
            nc.vector.tensor_tensor(eq[:], psl[:, :NE], mx[:].to_broadcast((P, NE)),
                                    op=mybir.AluOpType.is_equal)

            pst = ps_small.tile([P, P], f32, tag="pss")
            for hs in range(HSUB):
                nc.tensor.matmul(pst[:, :ER], xb[:, hs, :], loraAT_sb[:, hs, :],
                                 start=(hs == 0), stop=(hs == HSUB - 1))
            tmask = small.tile([P, 4, 16], bf16, tag="tmask")
            nc.vector.tensor_tensor(
                tmask[:],
                pst[:, :ER].rearrange("p (e r) -> p e r", r=16),
                eq[:, 1:NE, None].to_broadcast((P, 4, 16)),
                op=mybir.AluOpType.mult,
            )
            pstr = ps_small.tile([P, P], bf16, tag="pss")
            nc.tensor.transpose(pstr[:ER, :], tmask[:].rearrange("p e r -> p (e r)"),
                                ident[:])
            nc.vector.tensor_copy(tmaskT_sb[:ER, tt, :], pstr[:ER, :])

        # ---- Phase B: GeGLU base MLP (bf16) + LoRA output, two token passes
        for tp in range(NPASS):
            xt = xpool.tile([P, HSUB, TPASS], bf16, tag="xt")
            nc.sync.dma_start(xt[:], xT16_r[:, :, tp * TPASS:(tp + 1) * TPASS])
            out_sb = opool.tile([P, TLP, H], f32, tag="out_sb")

            for ic in range(NIC):
                gt = wpool.tile([P, HSUB, ICH], bf16, tag="gt")
                nc.sync.dma_start(gt[:], gateT_r[:, :, ic * ICH:(ic + 1) * ICH])
                ut = wpool.tile([P, HSUB, ICH], bf16, tag="ut")
                nc.sync.dma_start(ut[:], upT_r[:, :, ic * ICH:(ic + 1) * ICH])
                dt_ = wpool.tile([P, ITC, H], bf16, tag="dt", bufs=1)
                nc.sync.dma_start(dt_[:], downT_r[:, ic * ITC:(ic + 1) * ITC, :])

                hh = hpool.tile([P, ITC, TPASS], bf16, tag="hh")
                for it in range(ITC):
                    psg = ps_g.tile([P, TPASS], f32, tag="psg")
                    for hs in range(HSUB):
                        nc.tensor.matmul(psg[:], gt[:, hs, it * P:(it + 1) * P],
                                         xt[:, hs, :],
                                         start=(hs == 0), stop=(hs == HSUB - 1))
                    psu = ps_u.tile([P, TPASS], f32, tag="psu")
                    for hs in range(HSUB):
                        nc.tensor.matmul(psu[:], ut[:, hs, it * P:(it + 1) * P],
                                         xt[:, hs, :],
                                         start=(hs == 0), stop=(hs == HSUB - 1))
                    ga = apool.tile([P, TPASS], bf16, tag="ga")
                    nc.scalar.activation(ga[:], psg[:], gelu)
                    nc.vector.tensor_mul(hh[:, it, :], ga[:], psu[:])

                for tl in range(TLP):
                    for hc in range(NHC):
                        pso = ps_o.tile([P, 512], f32, tag="pso")
                        for it in range(ITC):
                            nc.tensor.matmul(pso[:], hh[:, it, tl * P:(tl + 1) * P],
                                             dt_[:, it, hc * 512:(hc + 1) * 512],
                                             start=(it == 0), stop=(it == ITC - 1))
                        osl = out_sb[:, tl, hc * 512:(hc + 1) * 512]
                        if ic == 0:
                            nc.scalar.copy(osl, pso[:])
                        else:
                            nc.vector.tensor_add(osl, osl, pso[:])

            for tl in range(TLP):
                tt = tp * TLP + tl
                for hc in range(NHC):
                    pso = ps_o.tile([P, 512], f32, tag="pso")
                    nc.tensor.matmul(pso[:], tmaskT_sb[:, tt, :],
                                     bflat_sb[:, hc * 512:(hc + 1) * 512],
                                     start=True, stop=True)
                    osl = out_sb[:, tl, hc * 512:(hc + 1) * 512]
                    nc.vector.tensor_add(osl, osl, pso[:])
                nc.sync.dma_start(out_r[:, tp * TLP + tl, :], out_sb[:, tl, :])

    nc.compile()
    return nc


def _get_nc():
    if "nc" not in _CACHE:
        _CACHE["nc"] = _build()
    return _CACHE["nc"]


def _prep_inputs(hidden_states, router_w, gate_w, up_w, down_w, lora_A, lora_B):
    x = np.ascontiguousarray(hidden_states, dtype=np.float32).reshape(NT, H)

    gateT = np.ascontiguousarray(gate_w.astype(bf16_np).T)      # [H, I]
    upT = np.ascontiguousarray(up_w.astype(bf16_np).T)          # [H, I]
    downT = np.ascontiguousarray(down_w.astype(bf16_np).T)      # [I, H]
    loraAT = np.ascontiguousarray(
        lora_A.reshape(ER, H).astype(bf16_np).T)                # [H, 64]
    bflat = np.zeros((P, H), dtype=bf16_np)
    bflat[:ER] = (lora_B.transpose(0, 2, 1).reshape(ER, H) * 2.0).astype(bf16_np)
    routerT = np.ascontiguousarray(router_w.T, dtype=np.float32)  # [H, 5]
    rhi = routerT.astype(bf16_np)
    rlo = (routerT - rhi.astype(np.float32)).astype(bf16_np)

    in_maps = []
    for c in range(NCORES):
        xc = np.ascontiguousarray(x[c * TC:(c + 1) * TC].T)     # [H, TC]
        xhi = xc.astype(bf16_np)
        xlo = (xc - xhi.astype(np.float32)).astype(bf16_np)
        in_maps.append({
            "xT16": xhi,
            "xloT": xlo,
            "gateT": gateT,
            "upT": upT,
            "downT": downT,
            "loraAT": loraAT,
            "bflat": bflat,
            "rhi": rhi,
            "rlo": rlo,
        })
    return in_maps


def run(inputs, trace=False, **trace_kwargs):
    """Compile (cached), shard, run on 8 cores; returns (output, aux_loss, raw)."""
    nc = _get_nc()
    in_maps = _prep_inputs(**inputs)
    raw = run_bass_kernel_spmd(nc, in_maps, list(range(NCORES)), trace=trace,
                               **trace_kwargs)

    out = np.empty((NT, H), dtype=np.float32)
    logits = np.empty((NT, NE), dtype=np.float32)
    for c, r in enumerate(raw.results):
        out[c * TC:(c + 1) * TC] = r["out"]
        logits[c * TC:(c + 1) * TC] = r["logits"]

    # aux loss from device-computed fp32 logits (tiny O(NT*E) reduction)
    m = logits.max(axis=1, keepdims=True)
    p = np.exp(logits - m)
    p /= p.sum(axis=1, keepdims=True)
    expert_probs = p.mean(axis=0)                                # [5]
    sel = logits.argmax(axis=1)
    counts = np.bincount(sel, minlength=NE).astype(np.float32) / NT
    aux = np.float32((expert_probs * counts).sum() * NE * 0.01)

    return out.reshape(B, S, H), aux, raw


def kernel(**inputs):
    output, aux, _ = run(inputs, trace=False)
    return output, aux


# revision 12
# speedup vs baseline: 1.0188x; 1.0177x over previous
"""Trainium2 Bass kernel for MoE-MLP (shared GeGLU base + top-1 LoRA experts).

Sharding: tokens (B*S = 8192) split evenly across 8 NeuronCores (1024 each);
MLP weights replicated per core (streamed from HBM, bf16). Router runs in
fp32 on-device for exact top-1; the tiny softmax/aux-loss reduction is
finished on host from the device-computed logits during unsharding.
"""

import numpy as np
import ml_dtypes
from contextlib import ExitStack

import concourse.bass as bass
import concourse.mybir as mybir
from concourse import bacc
from concourse.tile import TileContext
from concourse.bass_utils import run_bass_kernel_spmd
from concourse.masks import make_identity

P = 128
B, S, H, I = 4, 2048, 2048, 16384
NE = 5               # experts (1 base + 4 lora)
ER = 64              # lora (expert, rank) flattened
NT = B * S           # 8192 tokens
NCORES = 8
TC = NT // NCORES    # 1024 tokens per core
TPASS = 512          # tokens per pass (SBUF budget)
NPASS = TC // TPASS  # 2
ICH = 512            # intermediate-dim chunk
NIC = I // ICH       # 32
HSUB = H // P        # 16
NTT = TC // P        # 8 token tiles per core
TLP = TPASS // P     # 4 token tiles per pass
ITC = ICH // P       # 4 i-subtiles per chunk
NHC = H // 512       # 4 output column chunks

f32 = mybir.dt.float32
bf16 = mybir.dt.bfloat16
bf16_np = ml_dtypes.bfloat16

_CACHE = {}


def _build():
    nc = bacc.Bacc("TRN2", target_bir_lowering=False, debug=False)

    xT16 = nc.declare_dram_parameter("xT16", [H, TC], bf16, isOutput=False)
    xloT = nc.declare_dram_parameter("xloT", [H, TC], bf16, isOutput=False)
    gateT = nc.declare_dram_parameter("gateT", [H, I], bf16, isOutput=False)
    upT = nc.declare_dram_parameter("upT", [H, I], bf16, isOutput=False)
    downT = nc.declare_dram_parameter("downT", [I, H], bf16, isOutput=False)
    loraAT = nc.declare_dram_parameter("loraAT", [H, ER], bf16, isOutput=False)
    bflat = nc.declare_dram_parameter("bflat", [P, H], bf16, isOutput=False)
    rhi = nc.declare_dram_parameter("rhi", [H, NE], bf16, isOutput=False)
    rlo = nc.declare_dram_parameter("rlo", [H, NE], bf16, isOutput=False)
    out = nc.declare_dram_parameter("out", [TC, H], f32, isOutput=True)
    logits = nc.declare_dram_parameter("logits", [TC, NE], f32, isOutput=True)

    xT16_r = xT16.rearrange("(ho hi) t -> hi ho t", hi=P)      # [128,16,1024]
    xloT_r = xloT.rearrange("(ho hi) t -> hi ho t", hi=P)
    gateT_r = gateT.rearrange("(ho hi) i -> hi ho i", hi=P)    # [128,16,16384]
    upT_r = upT.rearrange("(ho hi) i -> hi ho i", hi=P)
    downT_r = downT.rearrange("(io ii) h -> ii io h", ii=P)    # [128,128,2048]
    loraAT_r = loraAT.rearrange("(ho hi) e -> hi ho e", hi=P)  # [128,16,64]
    rhi_r = rhi.rearrange("(ho hi) e -> hi ho e", hi=P)
    rlo_r = rlo.rearrange("(ho hi) e -> hi ho e", hi=P)
    out_r = out.rearrange("(to ti) h -> ti to h", ti=P)        # [128,8,2048]
    logits_r = logits.rearrange("(to ti) e -> ti to e", ti=P)  # [128,8,5]

    gelu = mybir.ActivationFunctionType.Gelu_apprx_tanh

    with TileContext(nc) as tc, ExitStack() as ctx:
        const = ctx.enter_context(tc.tile_pool(name="const", bufs=1))
        rpool = ctx.enter_context(tc.tile_pool(name="rpool", bufs=2))
        small = ctx.enter_context(tc.tile_pool(name="small", bufs=3))
        xpool = ctx.enter_context(tc.tile_pool(name="xpool", bufs=2))
        wpool = ctx.enter_context(tc.tile_pool(name="wpool", bufs=2))
        hpool = ctx.enter_context(tc.tile_pool(name="hpool", bufs=2))
        apool = ctx.enter_context(tc.tile_pool(name="apool", bufs=3))
        opool = ctx.enter_context(tc.tile_pool(name="opool", bufs=1))
        ps_small = ctx.enter_context(tc.tile_pool(name="ps_small", bufs=2, space="PSUM"))
        ps_g = ctx.enter_context(tc.tile_pool(name="ps_g", bufs=2, space="PSUM"))
        ps_u = ctx.enter_context(tc.tile_pool(name="ps_u", bufs=2, space="PSUM"))
        ps_o = ctx.enter_context(tc.tile_pool(name="ps_o", bufs=2, space="PSUM"))

        ident = const.tile([P, P], bf16)
        make_identity(nc, ident)

        rhi_sb = const.tile([P, HSUB, NE], bf16)
        nc.sync.dma_start(rhi_sb[:], rhi_r[:, :, :])
        rlo_sb = const.tile([P, HSUB, NE], bf16)
        nc.sync.dma_start(rlo_sb[:], rlo_r[:, :, :])
        loraAT_sb = const.tile([P, HSUB, ER], bf16)
        nc.sync.dma_start(loraAT_sb[:], loraAT_r[:, :, :])
        bflat_sb = const.tile([P, H], bf16)
        nc.sync.dma_start(bflat_sb[:], bflat[:, :])
        tmaskT_sb = const.tile([P, NTT, P], bf16)
        nc.vector.memset(tmaskT_sb[:], 0.0)

        # Phase A work (router via compensated bf16: xb@rhi + xb@rlo + xlo@rhi,
        # logits error ~1e-5 => argmax matches fp32; masked LoRA intermediate).
        # Issued interleaved into phase B's chunk loop so PE never stalls on
        # the DMA->router->DVE->transpose latency chain.
        tmasks = {}

        def phase_a_mm(tt):
            xb = rpool.tile([P, HSUB, P], bf16, tag="xb")
            nc.sync.dma_start(xb[:], xT16_r[:, :, tt * P:(tt + 1) * P])
            xlo = rpool.tile([P, HSUB, P], bf16, tag="xlo")
            nc.sync.dma_start(xlo[:], xloT_r[:, :, tt * P:(tt + 1) * P])

            psl = ps_small.tile([P, P], f32, tag="pss")
            for hs in range(HSUB):
                nc.tensor.matmul(psl[:, :NE], xb[:, hs, :], rhi_sb[:, hs, :],
                                 start=(hs == 0), stop=False)
                nc.tensor.matmul(psl[:, :NE], xb[:, hs, :], rlo_sb[:, hs, :],
                                 start=False, stop=False)
                nc.tensor.matmul(psl[:, :NE], xlo[:, hs, :], rhi_sb[:, hs, :],
                                 start=False, stop=(hs == HSUB - 1))
            lsb = small.tile([P, NE], f32, tag="lsb")
            nc.scalar.copy(lsb[:], psl[:, :NE])
            nc.sync.dma_start(logits_r[:, tt, :], lsb[:])
            mx = small.tile([P, 1], f32, tag="mx")
            nc.vector.reduce_max(mx[:], psl[:, :NE], axis=mybir.AxisListType.X)
            eq = small.tile([P, NE], bf16, tag="eq")
            nc.vector.tensor_tensor(eq[:], psl[:, :NE], mx[:].to_broadcast((P, NE)),
                                    op=mybir.AluOpType.is_equal)

            pst = ps_small.tile([P, P], f32, tag="pss")
            for hs in range(HSUB):
                nc.tensor.matmul(pst[:, :ER], xb[:, hs, :], loraAT_sb[:, hs, :],
                                 start=(hs == 0), stop=(hs == HSUB - 1))
            tmask = small.tile([P, 4, 16], bf16, tag="tmask")
            nc.vector.tensor_tensor(
                tmask[:],
                pst[:, :ER].rearrange("p (e r) -> p e r", r=16),
                eq[:, 1:NE, None].to_broadcast((P, 4, 16)),
                op=mybir.AluOpType.mult,
            )
            tmasks[tt] = tmask

        def phase_a_tr(tt):
            pstr = ps_small.tile([P, P], bf16, tag="pss")
            nc.tensor.transpose(pstr[:ER, :],
                                tmasks.pop(tt).rearrange("p e r -> p (e r)"),
                                ident[:])
            nc.vector.tensor_copy(tmaskT_sb[:ER, tt, :], pstr[:ER, :])

        # First phase-A tile up front: its small DMAs land before the bulk
        # weight DMAs, so PE has router work within ~3us of kernel start.
        phase_a_mm(0)

        # ---- Phase B: GeGLU base MLP (bf16) + LoRA output, two token passes
        for tp in range(NPASS):
            xt = xpool.tile([P, HSUB, TPASS], bf16, tag="xt")
            xt_src = xT16_r[:, :, tp * TPASS:(tp + 1) * TPASS]
            out_sb = opool.tile([P, TLP, H], f32, tag="out_sb")

            for ic in range(NIC):
                gt = wpool.tile([P, HSUB, ICH], bf16, tag="gt")
                ut = wpool.tile([P, HSUB, ICH], bf16, tag="ut")
                gsrc = gateT_r[:, :, ic * ICH:(ic + 1) * ICH]
                usrc = upT_r[:, :, ic * ICH:(ic + 1) * ICH]
                if tp == 0 and ic == 0:
                    # critical-path order: interleave gate/x slices first,
                    # then up, then down (consumed progressively later)
                    for q in range(ITC):
                        sl = slice(P * q, P * (q + 1))
                        nc.sync.dma_start(gt[:, :, sl], gsrc[:, :, sl])
                        nc.sync.dma_start(xt[:, 4 * q:4 * (q + 1), :],
                                          xt_src[:, 4 * q:4 * (q + 1), :])
                    for q in range(ITC):
                        sl = slice(P * q, P * (q + 1))
                        nc.sync.dma_start(ut[:, :, sl], usrc[:, :, sl])
                else:
                    if ic == 0:
                        nc.sync.dma_start(xt[:], xt_src)
                    nc.sync.dma_start(gt[:], gsrc)
                    nc.sync.dma_start(ut[:], usrc)
                dt_ = wpool.tile([P, ITC, H], bf16, tag="dt", bufs=1)
                nc.sync.dma_start(dt_[:], downT_r[:, ic * ITC:(ic + 1) * ITC, :])

                if tp == 0:
                    if 1 <= ic <= NTT:
                        phase_a_tr(ic - 1)
                    if ic < NTT - 1:
                        phase_a_mm(ic + 1)

                hh = hpool.tile([P, ITC, TPASS], bf16, tag="hh")
                for it in range(ITC):
                    psg = ps_g.tile([P, TPASS], f32, tag="psg")
                    for hs in range(HSUB):
                        nc.tensor.matmul(psg[:], gt[:, hs, it * P:(it + 1) * P],
                                         xt[:, hs, :],
                                         start=(hs == 0), stop=(hs == HSUB - 1))
                    psu = ps_u.tile([P, TPASS], f32, tag="psu")
                    for hs in range(HSUB):
                        nc.tensor.matmul(psu[:], ut[:, hs, it * P:(it + 1) * P],
                                         xt[:, hs, :],
                                         start=(hs == 0), stop=(hs == HSUB - 1))
                    ga = apool.tile([P, TPASS], bf16, tag="ga")
                    nc.scalar.activation(ga[:], psg[:], gelu)
                    nc.vector.tensor_mul(hh[:, it, :], ga[:], psu[:])

                for tl in range(TLP):
                    for hc in range(NHC):
                        pso = ps_o.tile([P, 512], f32, tag="pso")
                        for it in range(ITC):
                            nc.tensor.matmul(pso[:], hh[:, it, tl * P:(tl + 1) * P],
                                             dt_[:, it, hc * 512:(hc + 1) * 512],
                                             start=(it == 0), stop=(it == ITC - 1))
                        osl = out_sb[:, tl, hc * 512:(hc + 1) * 512]
                        if ic == 0:
                            nc.scalar.copy(osl, pso[:])
                        else:
                            nc.vector.tensor_add(osl, osl, pso[:])

            for tl in range(TLP):
                tt = tp * TLP + tl
                for hc in range(NHC):
                    pso = ps_o.tile([P, 512], f32, tag="pso")
                    nc.tensor.matmul(pso[:], tmaskT_sb[:, tt, :],
                                     bflat_sb[:, hc * 512:(hc + 1) * 512],
                                     start=True, stop=True)
                    osl = out_sb[:, tl, hc * 512:(hc + 1) * 512]
                    nc.vector.tensor_add(osl, osl, pso[:])
                    nc.sync.dma_start(out_r[:, tt, hc * 512:(hc + 1) * 512], osl)

    nc.compile()
    return nc


def _get_nc():
    if "nc" not in _CACHE:
        _CACHE["nc"] = _build()
    return _CACHE["nc"]


def _prep_inputs(hidden_states, router_w, gate_w, up_w, down_w, lora_A, lora_B):
    x = np.ascontiguousarray(hidden_states, dtype=np.float32).reshape(NT, H)

    gateT = np.ascontiguousarray(gate_w.astype(bf16_np).T)      # [H, I]
    upT = np.ascontiguousarray(up_w.astype(bf16_np).T)          # [H, I]
    downT = np.ascontiguousarray(down_w.astype(bf16_np).T)      # [I, H]
    loraAT = np.ascontiguousarray(
        lora_A.reshape(ER, H).astype(bf16_np).T)                # [H, 64]
    bflat = np.zeros((P, H), dtype=bf16_np)
    bflat[:ER] = (lora_B.transpose(0, 2, 1).reshape(ER, H) * 2.0).astype(bf16_np)
    routerT = np.ascontiguousarray(router_w.T, dtype=np.float32)  # [H, 5]
    rhi = routerT.astype(bf16_np)
    rlo = (routerT - rhi.astype(np.float32)).astype(bf16_np)

    in_maps = []
    for c in range(NCORES):
        xc = np.ascontiguousarray(x[c * TC:(c + 1) * TC].T)     # [H, TC]
        xhi = xc.astype(bf16_np)
        xlo = (xc - xhi.astype(np.float32)).astype(bf16_np)
        in_maps.append({
            "xT16": xhi,
            "xloT": xlo,
            "gateT": gateT,
            "upT": upT,
            "downT": downT,
            "loraAT": loraAT,
            "bflat": bflat,
            "rhi": rhi,
            "rlo": rlo,
        })
    return in_maps


def run(inputs, trace=False, **trace_kwargs):
    """Compile (cached), shard, run on 8 cores; returns (output, aux_loss, raw)."""
    nc = _get_nc()
    in_maps = _prep_inputs(**inputs)
    raw = run_bass_kernel_spmd(nc, in_maps, list(range(NCORES)), trace=trace,
                               **trace_kwargs)

    out = np.empty((NT, H), dtype=np.float32)
    logits = np.empty((NT, NE), dtype=np.float32)
    for c, r in enumerate(raw.results):
        out[c * TC:(c + 1) * TC] = r["out"]
        logits[c * TC:(c + 1) * TC] = r["logits"]

    # aux loss from device-computed fp32 logits (tiny O(NT*E) reduction)
    m = logits.max(axis=1, keepdims=True)
    p = np.exp(logits - m)
    p /= p.sum(axis=1, keepdims=True)
    expert_probs = p.mean(axis=0)                                # [5]
    sel = logits.argmax(axis=1)
    counts = np.bincount(sel, minlength=NE).astype(np.float32) / NT
    aux = np.float32((expert_probs * counts).sum() * NE * 0.01)

    return out.reshape(B, S, H), aux, raw


def kernel(**inputs):
    output, aux, _ = run(inputs, trace=False)
    return output, aux
